# revision 33
# baseline (speedup 1.0000x reference)
"""Trainium2 Bass kernel for nn_BCNLayer (locally-connected 7x7 lattice layer + sigmoid).

Math: y[i,j,b] = sigmoid( sum_{dy,dx in [-3,3]} w[dy+3,dx+3][(i-dy)*W + (j-dx)]
                          * x[(i-dy)*W + (j-dx), b] )   (zero outside lattice)

Strategy:
  - 8-way shard over lattice rows (H=128 -> 16 dest rows/core, 22 source rows
    with 3-row halos, zero-padded at the edges).
  - For one dest row i and source-row offset d (7 of them), the contribution is
    a banded 128x128 matrix (band +-3 over lattice columns) applied to the
    source row's [128 cols x B batch] slab:  out[jd, b] += sum_js
    Wband[js, jd] * x[js, b].  That is exactly nc.tensor.matmul(psum, lhsT=Wband,
    rhs=xrow) accumulated over the 7 source rows.
  - Banded matrices are prebuilt on the host (numpy) and DMA'd in; HW executes
    pure DMA + matmul + sigmoid.
  - DMA plan: x streams on the SP HWDGE ring in fine-grained pieces (chunk-0
    rows first, so the first matmul starts ~5us earlier); the banded weights
    stream on the ACT HWDGE ring in partition-major layout (>=1.75KB contiguous
    runs per descriptor, line-rate) with a tiny [t0,d0] piece first so the
    first LDWEIGHTS is never the gating input.  Output DMAs go on the gpsimd
    (SWDGE) ring, which is otherwise idle.
"""

import os

import numpy as np

H = 128
W = 128
HW = H * W
B = 1024
NCORES = 8
T = H // NCORES  # dest rows per core = 16
S = T + 6        # source rows per core (halo 3 each side) = 22
BC = 512         # batch chunk (fp32 psum bank = 512 fp32 per partition)
NB = B // BC     # chunks = 2

# dtype mode for the matmul inputs:
#   "f16"  - fp16 x and weights (10-bit mantissa, halves input traffic; fast)
#   "f32r" - tf32 path (10-bit mantissa products, fp32-sized traffic)
#   "f32"  - exact fp32 (4x slower matmul)
MM_MODE = os.environ.get("KERNEL_MM_MODE", "f16")
# output dtype: bf16 halves output traffic; adds <=2^-9 relative rounding
# (tolerance is 2e-2; f16 compute already sits at ~8e-3)
OUT_MODE = os.environ.get("KERNEL_OUT_MODE", "bf16")

_cache: dict = {}

# filled by the last kernel() call when KERNEL_TRACE=1
last_exec_time_ns = None
last_results = None

# Single-ring FIFO load schedule, ordered by first-use time: wb pieces are in
# units of 128x128 matrices (t*7+d flat), x pieces are (chunk, row_lo, row_hi).
# Interleaving on ONE HWDGE ring guarantees x rows are never starved by the
# wb bulk (rings round-robin at packet granularity, so a second ring would
# steal half the bandwidth exactly when x c0 is critical-path).
# Few fat pieces, boundaries aligned to first-use times (supply rate 436GB/s
# beats the 238GB/s demand rate, so only the t0/t1 startup lump gates; every
# extra piece costs ~0.65us of serialized issue + ~1us completion latency).
LOAD_SCHEDULE = [
    ("wb", 0, 7),        # t0 weights
    ("x", 0, 0, 8),      # rows 0-7: covers dest rows t0 AND t1
    ("wb", 7, 28),       # t1-t3
    ("x", 0, 8, 13),
    ("wb", 28, 56),      # t4-t7
    ("x", 0, 13, 18),
    ("wb", 56, 112),     # t8-t15
    ("x", 0, 18, 22),
    ("x", 1, 0, 8),
    ("x", 1, 8, 15),
    ("x", 1, 15, 22),
]
NWARM = 110  # HAM-warmup matmuls (n=64, 53ns at the 1.2GHz mid-state): a
             # continuous ~5us chain that crosses the ~3.4us HAM upshift
             # threshold and keeps the PE busy until the first real matmul's
             # inputs land (~12.5us), so real matmuls run at 2.4 GHz from #1.
             # (n=4 warmups do NOT earn HAM ramp credit; n=512 bursts trip
             # the throttled P0 ladder; n=64 measurably stays on the normal
             # ladder.)


def _build_program(mode: str, out_mode: str):
    from contextlib import ExitStack

    import concourse.bacc as bacc
    import concourse.mybir as mybir
    import concourse.tile as tile

    nc = bacc.Bacc(
        "TRN2", target_bir_lowering=False, debug=False, num_devices=NCORES
    )
    mm_dt = {
        "f32": mybir.dt.float32,
        "f32r": mybir.dt.float32r,
        "f16": mybir.dt.float16,
    }[mode]
    out_dt = {
        "f32": mybir.dt.float32,
        "bf16": mybir.dt.bfloat16,
    }[out_mode]
    # x in partition-major chunk-major layout [p, c, s, bc]: a load piece
    # (c, lo:hi) is then (hi-lo)KB contiguous per partition on BOTH the DRAM
    # and SBUF side -> ~128 fat descriptors per piece instead of ~900 1KB
    # ones (the 1KB granularity capped the 16 SDMA engines at ~60% of line
    # rate and made the input stream the critical path of the kernel head).
    xs = nc.dram_tensor("xs", [128, NB, S, BC], mm_dt, kind="ExternalInput").ap()
    # p-major banded weights: [js (partition), t*7*128 flat (t, d, jd)]
    wb = nc.dram_tensor(
        "wb", [128, T * 7 * 128], mm_dt, kind="ExternalInput"
    ).ap()
    y = nc.dram_tensor(
        "y", [T, 128, B], out_dt, kind="ExternalOutput"
    ).ap()

    from concourse.tile_rust import add_dep_helper

    with tile.TileContext(nc) as tc, ExitStack() as ctx:
        xpool = ctx.enter_context(tc.tile_pool(name="x", bufs=1))
        wpool = ctx.enter_context(tc.tile_pool(name="w", bufs=1))
        ppool = ctx.enter_context(tc.tile_pool(name="ps", bufs=7, space="PSUM"))
        jpool = ctx.enter_context(tc.tile_pool(name="pj", bufs=1, space="PSUM"))
        opool = ctx.enter_context(tc.tile_pool(name="o", bufs=8))

        xt = xpool.tile([128, NB * S * BC], mm_dt, tag="xslab")
        wt = wpool.tile([128, T * 7 * 128], mm_dt, tag="wslab")

        # (A parallel scalar-ring DMA for wb[t0,d0] was tried and reverted:
        # its completion semaphore shares a lane with the big SP-ring pieces,
        # so its completion only becomes visible ~3.5us late.)

        # Warm the sigmoid ACT table during the load phase (it otherwise loads
        # lazily right before the first real sigmoid, stalling the pipeline).
        warm = opool.tile([128, 1], mybir.dt.float32, tag="warm")
        nc.vector.memset(warm[:], 0.0)
        nc.scalar.activation(warm[:], warm[:], mybir.ActivationFunctionType.Sigmoid)

        # HAM clock-ramp warmup: the tensor engine reaches 2.4 GHz only after
        # ~3.4us of continuous busy (two 4096-cycle observation windows).
        # Keep it "busy" through the load phase with tiny n=4 matmuls on junk
        # data: ~4% MAC duty, so unlike a full-width warmup burst (v3) it
        # does not push the chip into the throttled P0 ladder.
        junk = opool.tile([128, 128], mm_dt, tag="junk")
        nc.vector.memset(junk[:], 0.0)
        pjunk = jpool.tile([128, 64], mybir.dt.float32, tag="pjunk")
        for k in range(NWARM):
            nc.tensor.matmul(
                pjunk[:], junk[:], junk[:, 0:64],
                start=(k == 0), stop=(k == NWARM - 1),
            )

        xt3 = xt[:].rearrange("p (c s b) -> p c s b", c=NB, s=S)

        # All input pieces on the SP HWDGE ring in first-use order.
        last_load = None
        for piece in LOAD_SCHEDULE:
            if piece[0] == "wb":
                _, lo, hi = piece
                last_load = nc.sync.dma_start(
                    out=wt[:, lo * 128 : hi * 128],
                    in_=wb[:, lo * 128 : hi * 128],
                )
            else:
                _, c, lo, hi = piece
                last_load = nc.sync.dma_start(
                    out=xt3[:, c, lo:hi, :], in_=xs[:, c, lo:hi, :]
                )



        for c in range(NB):
            for t in range(T):
                # Split the very last tile into two half-batch pieces so its
                # sigmoid+store overlap the final matmuls instead of being
                # fully exposed in the tail.
                last = c == NB - 1 and t == T - 1
                halves = [(0, BC)] if not last else [(0, BC // 2), (BC // 2, BC)]
                for b0, b1 in halves:
                    # Separate psum tile per half: sharing one bank makes the
                    # second half's matmuls wait out the first half's sigmoid
                    # read (WAR hazard at bank granularity).
                    ps = ppool.tile([128, BC], mybir.dt.float32, tag="ps")
                    for d in range(7):
                        lhs = wt[:, (t * 7 + d) * 128 : (t * 7 + d + 1) * 128]
                        base = (c * S + t + d) * BC
                        nc.tensor.matmul(
                            ps[:, 0 : b1 - b0], lhs, xt[:, base + b0 : base + b1],
                            start=(d == 0), stop=(d == 6),
                        )
                    ot = opool.tile([128, b1 - b0], out_dt, tag=f"o{b1 - b0}")
                    nc.scalar.activation(
                        ot[:], ps[:, 0 : b1 - b0],
                        mybir.ActivationFunctionType.Sigmoid,
                    )
                    # Chunk-0 outputs ride the otherwise-idle gpsimd SWDGE
                    # ring (mid-kernel, completion latency irrelevant).
                    # Chunk-1 outputs ride the sync HWDGE ring for its ~3x
                    # lower completion latency at the tail -- but ordered
                    # explicitly behind the last input load: an out-issue
                    # blocks the sequencer on its sigmoid semaphore, and the
                    # scheduler otherwise interleaves out-issues before the
                    # c1 loads, starving the tensor engine at the c0->c1
                    # transition.
                    if c == 0:
                        nc.gpsimd.dma_start(
                            out=y[t, :, c * BC + b0 : c * BC + b1], in_=ot[:]
                        )
                    else:
                        od = nc.sync.dma_start(
                            out=y[t, :, c * BC + b0 : c * BC + b1], in_=ot[:]
                        )
                        add_dep_helper(
                            od.ins, last_load.ins, False,
                            "keep c1 outs behind input loads on SP ring",
                        )
    nc.compile()
    return nc


def _build_banded(weights: np.ndarray) -> np.ndarray:
    """G[i, d, js, jd] = weight of edge (src row i+d-3, col js) -> (dest row i, col jd).

    dy = 3 - d (dest = src + dy), dx = jd - js, weight index = w[dy+3, dx+3][src_hw].
    """
    w4 = weights.reshape(7, 7, H, W)
    G = np.zeros((H, 7, W, W), np.float32)
    i = np.arange(H)
    for d in range(7):
        r = i + d - 3
        vi = i[(r >= 0) & (r < H)]
        if len(vi) == 0:
            continue
        for dxi in range(7):
            dx = dxi - 3
            js = np.arange(max(0, -dx), W - max(0, dx))
            G[vi[:, None], d, js[None, :], js[None, :] + dx] = w4[6 - d, dxi][
                (vi + d - 3)[:, None], js[None, :]
            ]
    return G


def kernel(x: np.ndarray, weights: np.ndarray) -> np.ndarray:
    global last_exec_time_ns, last_results
    import ml_dtypes
    from concourse.bass_utils import run_bass_kernel_spmd

    x = np.ascontiguousarray(x, dtype=np.float32)
    weights = np.ascontiguousarray(weights, dtype=np.float32)

    key = (MM_MODE, OUT_MODE)
    if key not in _cache:
        _cache[key] = _build_program(MM_MODE, OUT_MODE)
    nc = _cache[key]

    io_dt = np.float16 if MM_MODE == "f16" else np.float32
    x3 = x.reshape(H, W, B)
    xp = np.zeros((H + 6, W, B), io_dt)
    xp[3 : H + 3] = x3.astype(io_dt)
    G = _build_banded(weights).astype(io_dt)

    in_maps = []
    for q in range(NCORES):
        # [s, p, b] -> partition-major chunk-major [p, c, s, bc]
        xq = xp[T * q : T * q + S]
        xq = xq.transpose(1, 0, 2).reshape(W, S, NB, BC).transpose(0, 2, 1, 3)
        # [t, d, js, jd] -> partition-major [js, t, d, jd]
        Gq = G[T * q : T * q + T].transpose(2, 0, 1, 3)
        in_maps.append(
            {
                "xs": np.ascontiguousarray(xq),
                "wb": np.ascontiguousarray(Gq.reshape(W, T * 7 * W)),
            }
        )

    trace = os.environ.get("KERNEL_TRACE", "0") == "1"
    res = run_bass_kernel_spmd(
        nc, in_maps, core_ids=list(range(NCORES)), trace=trace
    )
    last_exec_time_ns = res.exec_time_ns
    last_results = res
    out = np.concatenate(
        [np.asarray(r["y"]).astype(np.float32).reshape(T * W, B) for r in res.results],
        axis=0,
    )
    return out


# revision 35
# speedup vs baseline: 1.0386x; 1.0386x over previous
"""Trainium2 Bass kernel for nn_BCNLayer (locally-connected 7x7 lattice layer + sigmoid).

Math: y[i,j,b] = sigmoid( sum_{dy,dx in [-3,3]} w[dy+3,dx+3][(i-dy)*W + (j-dx)]
                          * x[(i-dy)*W + (j-dx), b] )   (zero outside lattice)

Strategy:
  - 8-way shard over lattice rows (H=128 -> 16 dest rows/core, 22 source rows
    with 3-row halos, zero-padded at the edges).
  - For one dest row i and source-row offset d (7 of them), the contribution is
    a banded 128x128 matrix (band +-3 over lattice columns) applied to the
    source row's [128 cols x B batch] slab:  out[jd, b] += sum_js
    Wband[js, jd] * x[js, b].  That is exactly nc.tensor.matmul(psum, lhsT=Wband,
    rhs=xrow) accumulated over the 7 source rows.
  - Banded matrices are prebuilt on the host (numpy) and DMA'd in; HW executes
    pure DMA + matmul + sigmoid.
  - DMA plan: x streams on the SP HWDGE ring in fine-grained pieces (chunk-0
    rows first, so the first matmul starts ~5us earlier); the banded weights
    stream on the ACT HWDGE ring in partition-major layout (>=1.75KB contiguous
    runs per descriptor, line-rate) with a tiny [t0,d0] piece first so the
    first LDWEIGHTS is never the gating input.  Output DMAs go on the gpsimd
    (SWDGE) ring, which is otherwise idle.
"""

import os

import numpy as np

H = 128
W = 128
HW = H * W
B = 1024
NCORES = 8
T = H // NCORES  # dest rows per core = 16
S = T + 6        # source rows per core (halo 3 each side) = 22
BC = 512         # batch chunk (fp32 psum bank = 512 fp32 per partition)
NB = B // BC     # chunks = 2

# dtype mode for the matmul inputs:
#   "f16"  - fp16 x and weights (10-bit mantissa, halves input traffic; fast)
#   "f32r" - tf32 path (10-bit mantissa products, fp32-sized traffic)
#   "f32"  - exact fp32 (4x slower matmul)
MM_MODE = os.environ.get("KERNEL_MM_MODE", "f16")
# output dtype: bf16 halves output traffic; adds <=2^-9 relative rounding
# (tolerance is 2e-2; f16 compute already sits at ~8e-3)
OUT_MODE = os.environ.get("KERNEL_OUT_MODE", "bf16")

_cache: dict = {}

# filled by the last kernel() call when KERNEL_TRACE=1
last_exec_time_ns = None
last_results = None

# Single-ring FIFO load schedule, ordered by first-use time: wb pieces are in
# units of 128x128 matrices (t*7+d flat), x pieces are (chunk, row_lo, row_hi).
# Interleaving on ONE HWDGE ring guarantees x rows are never starved by the
# wb bulk (rings round-robin at packet granularity, so a second ring would
# steal half the bandwidth exactly when x c0 is critical-path).
# Few fat pieces, boundaries aligned to first-use times (supply rate 436GB/s
# beats the 238GB/s demand rate, so only the t0/t1 startup lump gates; every
# extra piece costs ~0.65us of serialized issue + ~1us completion latency).
LOAD_SCHEDULE = [
    ("wb", 0, 7),        # t0 weights
    ("x", 0, 0, 8),      # rows 0-7: covers dest rows t0 AND t1
    ("wb", 7, 28),       # t1-t3
    ("x", 0, 8, 13),
    ("wb", 28, 56),      # t4-t7
    ("x", 0, 13, 18),
    ("wb", 56, 112),     # t8-t15
    ("x", 0, 18, 22),
    ("x", 1, 0, 8),
    ("x", 1, 8, 15),
    ("x", 1, 15, 22),
]
NWARM = 0    # PE warmup is a dead end (measured): the HAM clock ladder only
             # latches onto real n=512 matmul activity -- full 2.4GHz arrives
             # ~4-5us after the FIRST REAL matmul regardless of any junk
             # warmup chain (n=4 or n=64, 1-6us long), and n=512 warmup
             # bursts trip the throttled P0 ladder (2.0GHz for the whole
             # kernel).  A warmup chain only delays the real start.


def _build_program(mode: str, out_mode: str):
    from contextlib import ExitStack

    import concourse.bacc as bacc
    import concourse.mybir as mybir
    import concourse.tile as tile

    nc = bacc.Bacc(
        "TRN2", target_bir_lowering=False, debug=False, num_devices=NCORES
    )
    mm_dt = {
        "f32": mybir.dt.float32,
        "f32r": mybir.dt.float32r,
        "f16": mybir.dt.float16,
    }[mode]
    out_dt = {
        "f32": mybir.dt.float32,
        "bf16": mybir.dt.bfloat16,
    }[out_mode]
    # x in partition-major chunk-major layout [p, c, s, bc]: a load piece
    # (c, lo:hi) is then (hi-lo)KB contiguous per partition on BOTH the DRAM
    # and SBUF side -> ~128 fat descriptors per piece instead of ~900 1KB
    # ones (the 1KB granularity capped the 16 SDMA engines at ~60% of line
    # rate and made the input stream the critical path of the kernel head).
    xs = nc.dram_tensor("xs", [128, NB, S, BC], mm_dt, kind="ExternalInput").ap()
    # p-major banded weights: [js (partition), t*7*128 flat (t, d, jd)]
    wb = nc.dram_tensor(
        "wb", [128, T * 7 * 128], mm_dt, kind="ExternalInput"
    ).ap()
    y = nc.dram_tensor(
        "y", [T, 128, B], out_dt, kind="ExternalOutput"
    ).ap()

    from concourse.tile_rust import add_dep_helper

    with tile.TileContext(nc) as tc, ExitStack() as ctx:
        xpool = ctx.enter_context(tc.tile_pool(name="x", bufs=1))
        wpool = ctx.enter_context(tc.tile_pool(name="w", bufs=1))
        ppool = ctx.enter_context(tc.tile_pool(name="ps", bufs=7, space="PSUM"))
        jpool = ctx.enter_context(tc.tile_pool(name="pj", bufs=1, space="PSUM"))
        opool = ctx.enter_context(tc.tile_pool(name="o", bufs=8))

        xt = xpool.tile([128, NB * S * BC], mm_dt, tag="xslab")
        wt = wpool.tile([128, T * 7 * 128], mm_dt, tag="wslab")

        # (A parallel scalar-ring DMA for wb[t0,d0] was tried and reverted:
        # its completion semaphore shares a lane with the big SP-ring pieces,
        # so its completion only becomes visible ~3.5us late.)

        # Warm the sigmoid ACT table during the load phase (it otherwise loads
        # lazily right before the first real sigmoid, stalling the pipeline).
        warm = opool.tile([128, 1], mybir.dt.float32, tag="warm")
        nc.vector.memset(warm[:], 0.0)
        nc.scalar.activation(warm[:], warm[:], mybir.ActivationFunctionType.Sigmoid)

        # HAM clock-ramp warmup: the tensor engine reaches 2.4 GHz only after
        # ~3.4us of continuous busy (two 4096-cycle observation windows).
        # Keep it "busy" through the load phase with tiny n=4 matmuls on junk
        # data: ~4% MAC duty, so unlike a full-width warmup burst (v3) it
        # does not push the chip into the throttled P0 ladder.
        if NWARM:
            junk = opool.tile([128, 128], mm_dt, tag="junk")
            nc.vector.memset(junk[:], 0.0)
            pjunk = jpool.tile([128, 64], mybir.dt.float32, tag="pjunk")
            for k in range(NWARM):
                nc.tensor.matmul(
                    pjunk[:], junk[:], junk[:, 0:64],
                    start=(k == 0), stop=(k == NWARM - 1),
                )

        xt3 = xt[:].rearrange("p (c s b) -> p c s b", c=NB, s=S)

        # All input pieces on the SP HWDGE ring in first-use order.
        last_load = None
        for piece in LOAD_SCHEDULE:
            if piece[0] == "wb":
                _, lo, hi = piece
                last_load = nc.sync.dma_start(
                    out=wt[:, lo * 128 : hi * 128],
                    in_=wb[:, lo * 128 : hi * 128],
                )
            else:
                _, c, lo, hi = piece
                last_load = nc.sync.dma_start(
                    out=xt3[:, c, lo:hi, :], in_=xs[:, c, lo:hi, :]
                )



        for c in range(NB):
            for t in range(T):
                # Split the very last tile into two half-batch pieces so its
                # sigmoid+store overlap the final matmuls instead of being
                # fully exposed in the tail.
                last = c == NB - 1 and t == T - 1
                halves = [(0, BC)] if not last else [(0, BC // 2), (BC // 2, BC)]
                for b0, b1 in halves:
                    # Separate psum tile per half: sharing one bank makes the
                    # second half's matmuls wait out the first half's sigmoid
                    # read (WAR hazard at bank granularity).
                    ps = ppool.tile([128, BC], mybir.dt.float32, tag="ps")
                    for d in range(7):
                        lhs = wt[:, (t * 7 + d) * 128 : (t * 7 + d + 1) * 128]
                        base = (c * S + t + d) * BC
                        nc.tensor.matmul(
                            ps[:, 0 : b1 - b0], lhs, xt[:, base + b0 : base + b1],
                            start=(d == 0), stop=(d == 6),
                        )
                    ot = opool.tile([128, b1 - b0], out_dt, tag=f"o{b1 - b0}")
                    nc.scalar.activation(
                        ot[:], ps[:, 0 : b1 - b0],
                        mybir.ActivationFunctionType.Sigmoid,
                    )
                    # Chunk-0 outputs ride the otherwise-idle gpsimd SWDGE
                    # ring (mid-kernel, completion latency irrelevant).
                    # Chunk-1 outputs ride the sync HWDGE ring for its ~3x
                    # lower completion latency at the tail -- but ordered
                    # explicitly behind the last input load: an out-issue
                    # blocks the sequencer on its sigmoid semaphore, and the
                    # scheduler otherwise interleaves out-issues before the
                    # c1 loads, starving the tensor engine at the c0->c1
                    # transition.
                    if c == 0:
                        nc.gpsimd.dma_start(
                            out=y[t, :, c * BC + b0 : c * BC + b1], in_=ot[:]
                        )
                    else:
                        od = nc.sync.dma_start(
                            out=y[t, :, c * BC + b0 : c * BC + b1], in_=ot[:]
                        )
                        add_dep_helper(
                            od.ins, last_load.ins, False,
                            "keep c1 outs behind input loads on SP ring",
                        )
    nc.compile()
    return nc


def _build_banded(weights: np.ndarray) -> np.ndarray:
    """G[i, d, js, jd] = weight of edge (src row i+d-3, col js) -> (dest row i, col jd).

    dy = 3 - d (dest = src + dy), dx = jd - js, weight index = w[dy+3, dx+3][src_hw].
    """
    w4 = weights.reshape(7, 7, H, W)
    G = np.zeros((H, 7, W, W), np.float32)
    i = np.arange(H)
    for d in range(7):
        r = i + d - 3
        vi = i[(r >= 0) & (r < H)]
        if len(vi) == 0:
            continue
        for dxi in range(7):
            dx = dxi - 3
            js = np.arange(max(0, -dx), W - max(0, dx))
            G[vi[:, None], d, js[None, :], js[None, :] + dx] = w4[6 - d, dxi][
                (vi + d - 3)[:, None], js[None, :]
            ]
    return G


def kernel(x: np.ndarray, weights: np.ndarray) -> np.ndarray:
    global last_exec_time_ns, last_results
    import ml_dtypes
    from concourse.bass_utils import run_bass_kernel_spmd

    x = np.ascontiguousarray(x, dtype=np.float32)
    weights = np.ascontiguousarray(weights, dtype=np.float32)

    key = (MM_MODE, OUT_MODE)
    if key not in _cache:
        _cache[key] = _build_program(MM_MODE, OUT_MODE)
    nc = _cache[key]

    io_dt = np.float16 if MM_MODE == "f16" else np.float32
    x3 = x.reshape(H, W, B)
    xp = np.zeros((H + 6, W, B), io_dt)
    xp[3 : H + 3] = x3.astype(io_dt)
    G = _build_banded(weights).astype(io_dt)

    in_maps = []
    for q in range(NCORES):
        # [s, p, b] -> partition-major chunk-major [p, c, s, bc]
        xq = xp[T * q : T * q + S]
        xq = xq.transpose(1, 0, 2).reshape(W, S, NB, BC).transpose(0, 2, 1, 3)
        # [t, d, js, jd] -> partition-major [js, t, d, jd]
        Gq = G[T * q : T * q + T].transpose(2, 0, 1, 3)
        in_maps.append(
            {
                "xs": np.ascontiguousarray(xq),
                "wb": np.ascontiguousarray(Gq.reshape(W, T * 7 * W)),
            }
        )

    trace = os.environ.get("KERNEL_TRACE", "0") == "1"
    res = run_bass_kernel_spmd(
        nc, in_maps, core_ids=list(range(NCORES)), trace=trace
    )
    last_exec_time_ns = res.exec_time_ns
    last_results = res
    out = np.concatenate(
        [np.asarray(r["y"]).astype(np.float32).reshape(T * W, B) for r in res.results],
        axis=0,
    )
    return out
